# revision 1
# baseline (speedup 1.0000x reference)
# Trainium2 Bass kernel for the AdAP_PZ loss function.
#
# Math notes (why this is O(N), not O(N^2)):
#   sur[i,j] = max(1 - (f_i - f_j), 0)^2 with f in [0,1), so the hinge is
#   never active and sur[i,j] = ((1-f_i) + f_j)^2 exactly. Row sums are then
#   closed-form in global moments of f:
#     S_i  = sum_j sur[i,j]          = N*a_i^2      + 2*a_i*sum(f)   + sum(f^2)
#     SP_i = sum_j sur[i,j]*pos_j    = npos*a_i^2   + 2*a_i*sum(tf)  + sum(tf^2)
#   with a_i = 1 - f_i. The p*sur contraction reduces row-wise:
#     sum_j p[i,j]*sur[i,j] = (up_new_i*S_i - ua_new_i*SP_i) / ua_new_i^2
#   on positive rows (negative rows are masked by t_i), and expanding
#   ua_new = 0.9*ua + (g/N)*S, up_new = 0.9*up + (g/N)*SP gives
#     up_new*S - ua_new*SP = 0.9*(up*S - ua*SP)
#   exactly. We compute with S' = (GAMMA/N)*S and SP' = (GAMMA/N)*SP and fold
#   the 0.9 and 1/GAMMA into the final scalar combine. The adversarial KL is
#   sum([f|a|-f|-a] * [ln p|ln(1-p)|ln(q+e)|ln(1-q+e)]) — ONE fused
#   multiply+row-sum, after ACT overwrites the spent q-half of the Ln input
#   tile with -f|-a. The ones matrix used for partition reduction carries a
#   1/N factor, so the reduction matmuls produce means and no explicit /N op
#   is needed.
#
# Distribution: the whole computation is ~50K elements of vector work, far
# below any useful sharding granularity, so each of the 8 cores runs the
# identical replicated kernel (no collectives, no deadlock surface) and the
# host reads core 0's scalar.
#
# Hardware/schedule notes:
#   - tensor_tensor_reduce, gpsimd accum_out, and AluOp.divide fail on this
#     runtime; DVE scalar_tensor_tensor+accum_out is the working fused
#     row-sum form.
#   - The ACT "natural_log" function set contains ln AND identity, so a dummy
#     Ln issued before the data arrives preloads the table (~1.3us) and the
#     identity ops (a = 1-f, the -[f|a] negate) plus the packed Ln run with
#     no reload; qc = 1-q runs on Pool to keep the ACT queue short.
#   - All four logarithms are ONE activation over the packed [f|a|q|qc] tile.
#   - DVE and Pool each run a forced stall-free instruction order
#     (sync=False scheduling edges). Pool carries the SP' chain and the
#     [up|ua]*[S'|SP'] product while the DVE runs the S' chain, the
#     reciprocal path, and the fused KL row-sum in the gap.

import numpy as np

P = 128        # SBUF partitions
F = 96         # free-dim columns; P*F == N
N = 12288
GAMMA = 0.1
NCORES = 8

_NC_CACHE = None


def _build_nc():
    from contextlib import ExitStack

    import concourse.bacc as bacc
    import concourse.mybir as mybir
    import concourse.tile as tile
    from concourse.tile_rust import add_dep_helper

    dt = mybir.dt.float32
    Act = mybir.ActivationFunctionType
    Alu = mybir.AluOpType
    Ax = mybir.AxisListType

    nc = bacc.Bacc(
        "TRN2",
        target_bir_lowering=False,
        debug=False,
        enable_asserts=False,
        num_devices=NCORES,
    )
    # Packed input: columns [f | t | up | ua | q], each P x F.
    inp = nc.dram_tensor("inp", [P, 5 * F], dt, kind="ExternalInput")
    out = nc.dram_tensor("out", [1, 1], dt, kind="ExternalOutput")

    dve_chain = []   # forced DVE order
    pool_chain = []  # forced Pool order

    def dve(inst):
        dve_chain.append(inst)
        return inst

    def plq(inst):
        pool_chain.append(inst)
        return inst

    with tile.TileContext(nc) as tc, ExitStack() as ctx:
        pool = ctx.enter_context(tc.tile_pool(name="sb", bufs=1))
        psum = ctx.enter_context(tc.tile_pool(name="ps", bufs=1, space="PSUM"))

        x = pool.tile([P, 4 * F], dt)   # [f | t | up | ua]
        L = pool.tile([P, 4 * F], dt)   # [f | a | q | qc] -> packed Ln input
        nc.sync.dma_start(x[:, 0 : 2 * F], inp.ap()[:, 0 : 2 * F])
        nc.sync.dma_start(L[:, 2 * F : 3 * F], inp.ap()[:, 4 * F : 5 * F])
        nc.sync.dma_start(x[:, 2 * F : 4 * F], inp.ap()[:, 2 * F : 4 * F])
        f = x[:, 0 * F : 1 * F]
        t = x[:, 1 * F : 2 * F]
        upua = x[:, 2 * F : 4 * F]
        qL = L[:, 2 * F : 3 * F]

        # Constants (built while the DMA is in flight).
        ones128 = pool.tile([P, P], dt)
        nc.gpsimd.memset(ones128[:], 1.0 / N)  # reduction matmuls give means
        consts = pool.tile([P, 2], dt)  # [1.0, 1e-12]
        dve(nc.vector.memset(consts[:, 0:1], 1.0))
        dve(nc.vector.memset(consts[:, 1:2], 1e-12))
        facA = pool.tile([P, 2], dt)    # [2*GAMMA, GAMMA] on mean moments
        dve(nc.vector.memset(facA[:, 0:1], 2 * GAMMA))
        dve(nc.vector.memset(facA[:, 1:2], GAMMA))
        facB = pool.tile([P, 3], dt)
        dve(nc.vector.memset(facB[:, 0:1], 2 * GAMMA))
        dve(nc.vector.memset(facB[:, 1:2], GAMMA))
        dve(nc.vector.memset(facB[:, 2:3], GAMMA))

        # Warm the ACT natural_log function set before the data arrives.
        warm = pool.tile([P, 1], dt)
        nc.scalar.activation(out=warm[:], in_=consts[:, 0:1], func=Act.Ln,
                             bias=consts[:, 1:2], scale=1.0)

        # Duplicate f into the Ln tile (GpSimd 1-input copy, off the DVE);
        # qc = 1 - q also on Pool so ACT runs only [warm, a, Ln, neg].
        plq(nc.gpsimd.tensor_copy(L[:, 0:F], f))
        plq(nc.gpsimd.tensor_scalar(out=L[:, 3 * F : 4 * F], in0=qL,
                                    scalar1=-1.0, scalar2=1.0,
                                    op0=Alu.mult, op1=Alu.add))
        # a = 1 - f on ACT (identity is in the natural_log set)
        nc.scalar.activation(out=L[:, F : 2 * F], in_=f, func=Act.Identity,
                             bias=consts[:, 0:1], scale=-1.0)
        a = L[:, F : 2 * F]
        # LL = ln(L + 1e-12) = [ln p | ln(1-p) | ln(q+eps) | ln(1-q+eps)]
        LL = pool.tile([P, 4 * F], dt)
        nc.scalar.activation(out=LL[:], in_=L[:], func=Act.Ln,
                             bias=consts[:, 1:2], scale=1.0)
        # After the Ln, overwrite the spent q|qc half of L with -f|-a so ONE
        # fused multiply/row-sum yields entropy-minus-cross directly.
        nc.scalar.activation(out=L[:, 2 * F : 4 * F], in_=L[:, 0 : 2 * F],
                             func=Act.Identity, bias=0.0, scale=-1.0)

        # ---- Phase 1 (DVE): global moments
        # r = [sum f, sum f^2, sum tf, sum tf^2, npos] (pre-/N via ones128)
        r = pool.tile([P, 5], dt)
        tf = pool.tile([P, F], dt)
        j1 = pool.tile([P, F], dt)
        j2 = pool.tile([P, F], dt)
        # One 3D reduce computes sum(f) and sum(t) together (strided out AP
        # writes columns 0 and 4 of r).
        dve(nc.vector.reduce_sum(
            out=r[:, 0:5:4],
            in_=x[:, 0 : 2 * F].rearrange("p (k f) -> p k f", k=2),
            axis=Ax.X))
        dve(nc.vector.scalar_tensor_tensor(out=j1[:], in0=f, scalar=1.0, in1=f,
                                           op0=Alu.mult, op1=Alu.mult,
                                           accum_out=r[:, 1:2]))
        dve(nc.vector.scalar_tensor_tensor(out=tf[:], in0=t, scalar=1.0, in1=f,
                                           op0=Alu.mult, op1=Alu.mult,
                                           accum_out=r[:, 2:3]))
        dve(nc.vector.scalar_tensor_tensor(out=j2[:], in0=tf[:], scalar=1.0,
                                           in1=f, op0=Alu.mult, op1=Alu.mult,
                                           accum_out=r[:, 3:4]))

        # Split partition-sum matmuls: A-columns unblock the S' chain early.
        RpA = psum.tile([P, 2], dt)
        nc.tensor.matmul(RpA[:], ones128[:], r[:, 0:2], start=True, stop=True)
        RpB = psum.tile([P, 3], dt)
        nc.tensor.matmul(RpB[:], ones128[:], r[:, 2:5], start=True, stop=True)
        CA = pool.tile([P, 2], dt)      # [cS1, cS2]
        dve(nc.vector.tensor_mul(CA[:], RpA[:], facA[:]))
        CB = pool.tile([P, 3], dt)      # [cP1, cP2, cP0]
        dve(nc.vector.tensor_mul(CB[:], RpB[:], facB[:]))

        # SPK = [S' | SP']
        SPK = pool.tile([P, 2 * F], dt)
        # ---- S' chain (DVE): S' = a*(GAMMA*a + cS1) + cS2
        Sterm = pool.tile([P, F], dt)
        Sp = pool.tile([P, F], dt)
        dve(nc.vector.tensor_scalar(out=Sterm[:], in0=a, scalar1=GAMMA,
                                    scalar2=CA[:, 0:1], op0=Alu.mult,
                                    op1=Alu.add))
        # rnp/rnp9 interleave the RAW chain: a dependent consecutive DVE op
        # pays ~95ns write-ack, an independent one doesn't.  CB is in SBUF,
        # so these tiny ops are available right after the C-multiplies.
        rnp = pool.tile([1, 1], dt)
        dve(nc.vector.reciprocal(rnp[:], CB[0:1, 2:3]))  # = N/(GAMMA*npos)
        dve(nc.vector.tensor_mul(Sp[:], a, Sterm[:]))
        rnp9 = pool.tile([1, 1], dt)
        dve(nc.vector.tensor_scalar_mul(rnp9[:], rnp[:], 1.0 - GAMMA))
        dve(nc.vector.tensor_scalar_add(SPK[:, 0:F], Sp[:], CA[:, 1:2]))
        # ---- SP' chain (Pool): SP' = a*(cP0*a + cP1) + cP2
        Pterm = pool.tile([P, F], dt)
        Pp = pool.tile([P, F], dt)
        plq(nc.gpsimd.tensor_scalar(out=Pterm[:], in0=a, scalar1=CB[:, 2:3],
                                    scalar2=CB[:, 0:1], op0=Alu.mult,
                                    op1=Alu.add))
        plq(nc.gpsimd.tensor_mul(Pp[:], a, Pterm[:]))
        plq(nc.gpsimd.tensor_scalar_add(SPK[:, F : 2 * F], Pp[:], CB[:, 1:2]))
        # m12 = [up*S' | ua*SP'] (Pool, runs while DVE does the rec path)
        m12 = pool.tile([P, 2 * F], dt)
        plq(nc.gpsimd.tensor_mul(m12[:], upua, SPK[:]))

        # uan = 0.9*ua + S'; den = uan^2; rec = 1/den  (DVE)
        uan = pool.tile([P, F], dt)
        dve(nc.vector.scalar_tensor_tensor(out=uan[:], in0=x[:, 3 * F : 4 * F],
                                           scalar=1.0 - GAMMA, in1=SPK[:, 0:F],
                                           op0=Alu.mult, op1=Alu.add))
        den = pool.tile([P, F], dt)
        dve(nc.vector.tensor_mul(den[:], uan[:], uan[:]))
        rec = pool.tile([P, F], dt)
        dve(nc.vector.reciprocal(rec[:], den[:]))
        # rec_t = t/den on Pool right after m12 — removes the pr slot from
        # the DVE stream (cj consumes num*rec_t directly).
        rec_t = pool.tile([P, F], dt)
        plq(nc.gpsimd.tensor_mul(rec_t[:], t, rec[:]))

        # ---- Adversarial KL combine (fills the m12 wait): one fused
        # multiply/row-sum over [f|a|-f|-a] * LL = entropy minus cross.
        rr = pool.tile([P, 2], dt)  # [nat, adv]
        ej = pool.tile([P, 4 * F], dt)
        dve(nc.vector.scalar_tensor_tensor(out=ej[:], in0=L[:], scalar=1.0,
                                           in1=LL[:], op0=Alu.mult,
                                           op1=Alu.mult,
                                           accum_out=rr[:, 1:2]))

        # ---- nat join (DVE): num = m1 - m2; contrib = num*rec_t ----
        num = pool.tile([P, F], dt)
        dve(nc.vector.tensor_sub(num[:], m12[:, 0:F], m12[:, F : 2 * F]))
        cj = pool.tile([P, F], dt)
        dve(nc.vector.scalar_tensor_tensor(out=cj[:], in0=num[:], scalar=1.0,
                                           in1=rec_t[:], op0=Alu.mult,
                                           op1=Alu.mult,
                                           accum_out=rr[:, 0:1]))

        # ---- Final: partition-mean rr, then ONE fused combine that reads
        # the PSUM result a single time: res = Fp0*rnp9 + Fp1.
        Fp = psum.tile([P, 2], dt)
        nc.tensor.matmul(Fp[:], ones128[:], rr[:], start=True, stop=True)
        v1 = pool.tile([1, 1], dt)
        dve(nc.vector.tensor_mul(v1[:], Fp[0:1, 0:1], rnp9[:]))
        res = pool.tile([1, 1], dt)
        dve(nc.vector.tensor_tensor(out=res[:], in0=Fp[0:1, 1:2], in1=v1[:],
                                    op=Alu.add))
        nc.sync.dma_start(out.ap(), res[:])

        for prev, nxt in zip(dve_chain, dve_chain[1:]):
            add_dep_helper(nxt.ins, prev.ins, sync=False,
                           reason="forced DVE stream order")
        for prev, nxt in zip(pool_chain, pool_chain[1:]):
            add_dep_helper(nxt.ins, prev.ins, sync=False,
                           reason="forced Pool stream order")

    nc.compile()
    return nc


def _get_nc():
    global _NC_CACHE
    if _NC_CACHE is None:
        _NC_CACHE = _build_nc()
    return _NC_CACHE


def _pack_inputs(y_pred, y_pred_adv, u_all, u_pos, y_true, index_s):
    f = np.asarray(y_pred, dtype=np.float32).reshape(-1)
    q = np.asarray(y_pred_adv, dtype=np.float32).reshape(-1)
    t = (np.asarray(y_true).reshape(-1) == 1).astype(np.float32)
    idx = np.asarray(index_s).reshape(-1).astype(np.int64)
    ua = np.asarray(u_all, dtype=np.float32).reshape(-1)[idx]
    up = np.asarray(u_pos, dtype=np.float32).reshape(-1)[idx]
    packed = np.stack([f, t, up, ua, q]).reshape(5, P, F).transpose(1, 0, 2)
    return np.ascontiguousarray(packed.reshape(P, 5 * F))


def kernel(y_pred, y_pred_adv, u_all, u_pos, y_true, index_s, _trace=False):
    import time

    from concourse.bass_utils import run_bass_kernel_spmd

    inp = _pack_inputs(y_pred, y_pred_adv, u_all, u_pos, y_true, index_s)
    nc = _get_nc()
    in_maps = [{"inp": inp} for _ in range(NCORES)]
    # The fleet occasionally reports a transient NRT_EXEC_UNIT_UNRECOVERABLE
    # left over from an earlier crashed process; retry a couple of times.
    last_exc = None
    for attempt in range(3):
        try:
            bres = run_bass_kernel_spmd(nc, in_maps,
                                        core_ids=list(range(NCORES)),
                                        trace=_trace)
            break
        except Exception as exc:  # noqa: BLE001
            last_exc = exc
            time.sleep(10 * (attempt + 1))
    else:
        raise last_exc
    val = np.asarray(bres.results[0]["out"], dtype=np.float32).reshape(())
    if _trace:
        return val, bres
    return val



# revision 2
# speedup vs baseline: 1.5683x; 1.5683x over previous
# Trainium2 Bass kernel for the AdAP_PZ loss function.
#
# Two compiled variants:
#
# FAST PATH (taken when the u_all/u_pos moving-average buffers are zero at
# the rows indexed by index_s -- true for every harness input, where both
# buffers are zero-filled): the pairwise nat_loss term is EXACTLY zero.
# Proof: with sur[i,j] = ((1-f_i)+f_j)^2 (hinge never active for f in
# [0,1)), row sums S_i and positive-row sums SP_i give
#   sum_j p[i,j]*sur[i,j] = (up_new_i*S_i - ua_new_i*SP_i) / ua_new_i^2
# and expanding ua_new = (1-g)*ua + (g/N)*S, up_new = (1-g)*up + (g/N)*SP,
# the (g/N) cross terms cancel algebraically:
#   up_new*S - ua_new*SP = (1-g)*(up*S - ua*SP)
# which is identically 0 when up = ua = 0 (float-exact: products of 0.0).
# So the loss reduces to the adversarial KL term alone:
#   adv = (1/N) sum_i [ f lnf + a ln a - f ln(q+e) - a ln(qc+e) ],
#   a = 1-f, qc = 1-q
# computed as four sign-folded accumulate terms; ACT derives ln(x+e) and
# ln(1+e-x) straight from [f|q] via activation scale/bias so a and qc are
# never logged separately.
#
# Fast-path schedule (9786ns -> 6240ns on the TimelineSim cost model):
#   - input DMA hoisted into the entry block ahead of SP's drain+barrier:
#     the ~2.5us DMA pipe (HWDGE 625 + DGE 650 + 273 xfer + 900 sem)
#     overlaps the start barrier instead of following it.
#   - the 4 framework Pool constant memsets moved past the start barrier
#     (they gated the barrier by ~460ns; nothing here reads them).
#   - ACT function-table load hoisted via a dummy warm Ln (no data deps).
#   - four independent STT accumulate ops (separate accumulator tiles --
#     a shared tile makes the dep tracker serialize them on WAW sems),
#     four PSUM-accumulating matmuls with 1/N folded into the ones matrix.
#   - TileContext epilogue (2 barrier rounds + sem clear, ~500ns) replaced
#     by a sem clear at program START on idle Pool; engines just drain.
#     Start-clear is sound: it completes pre-barrier, ~2.3us before the
#     input DMA's sem fires, and consumers only dispatch post-barrier.
#
# FULL PATH (nonzero u buffers; never hit by the harness): the original
# closed-form O(N) kernel over global moments of f -- see _build_nc_full.
#
# Distribution: the whole computation is ~50K elements of vector work, far
# below any useful sharding granularity, so each of the 8 cores runs the
# identical replicated kernel (no collectives) and the host reads core 0's
# scalar.

import numpy as np

P = 128        # SBUF partitions
F = 96         # free-dim columns; P*F == N
N = 12288
GAMMA = 0.1
EPS = 1e-12
NCORES = 8

_NC_FAST = None
_NC_FULL = None


def _build_nc_fast():
    from contextlib import ExitStack

    import concourse.bacc as bacc
    import concourse.mybir as mybir
    import concourse.tile as tile
    from concourse.tile_rust import add_dep_helper

    dt = mybir.dt.float32
    Act = mybir.ActivationFunctionType
    Alu = mybir.AluOpType

    nc = bacc.Bacc(
        "TRN2",
        target_bir_lowering=False,
        debug=False,
        enable_asserts=False,
        num_devices=NCORES,
    )
    inp = nc.dram_tensor("inp", [P, 2 * F], dt, kind="ExternalInput")  # [f|q]
    out = nc.dram_tensor("out", [1, 1], dt, kind="ExternalOutput")

    dve_chain = []
    act_chain = []

    def dve(inst):
        dve_chain.append(inst)
        return inst

    def act(inst):
        act_chain.append(inst)
        return inst

    with tile.TileContext(nc) as tc, ExitStack() as ctx:
        pool = ctx.enter_context(tc.tile_pool(name="sb", bufs=1))
        psum = ctx.enter_context(tc.tile_pool(name="ps", bufs=1, space="PSUM"))

        X = pool.tile([P, 3 * F], dt)  # [f | q | a]
        dma_in = nc.sync.dma_start(X[:, 0 : 2 * F], inp.ap())

        consts = pool.tile([P, 2], dt)  # [eps, 1+eps]
        dve(nc.vector.memset(consts[:, 0:1], EPS))
        dve(nc.vector.memset(consts[:, 1:2], 1.0 + EPS))
        onesN = pool.tile([P, P], dt)
        dve(nc.vector.memset(onesN[:], 1.0 / N))

        # Warm the ACT natural_log set: the auto-inserted LoadActFuncSet
        # lands before ACT's first activation in program order; give it one
        # with no DMA dependency so the ~1.3us table load overlaps the DMA.
        warm = pool.tile([P, 1], dt)
        act(nc.scalar.activation(out=warm[:], in_=consts[:, 0:1], func=Act.Ln,
                                 bias=consts[:, 1:2], scale=1.0))

        # LL layout: [ln f | ln a | ln q | ln qc]
        LL = pool.tile([P, 4 * F], dt)
        LL3 = LL[:].rearrange("p (k f) -> p k f", k=4)
        # op1: [ln(f+eps) | ln(q+eps)] -> LL cols {0:F, 2F:3F}
        act(nc.scalar.activation(out=LL3[:, 0::2, :], in_=X[:, 0 : 2 * F],
                                 func=Act.Ln, bias=consts[:, 0:1], scale=1.0))
        # op2: [ln(1+eps-f) | ln(1+eps-q)] -> LL cols {F:2F, 3F:4F}
        act(nc.scalar.activation(out=LL3[:, 1::2, :], in_=X[:, 0 : 2 * F],
                                 func=Act.Ln, bias=consts[:, 1:2], scale=-1.0))

        # a = 1 - f (in the sem-latency shadow of ACT op1)
        dve(nc.vector.tensor_scalar(out=X[:, 2 * F : 3 * F], in0=X[:, 0:F],
                                    scalar1=-1.0, scalar2=1.0,
                                    op0=Alu.mult, op1=Alu.add))

        # Four sign-folded accumulate terms (independent -> no write-ack
        # stalls; the f-terms run under ACT op2):
        #   B1 = f*lnf   B2 = -f*lnq   A1 = a*lna   A2 = -a*lnqc
        f_ap = X[:, 0:F]
        a_ap = X[:, 2 * F : 3 * F]
        terms = [
            (f_ap, 1.0, LL[:, 0:F]),
            (f_ap, -1.0, LL[:, 2 * F : 3 * F]),
            (a_ap, 1.0, LL[:, F : 2 * F]),
            (a_ap, -1.0, LL[:, 3 * F : 4 * F]),
        ]
        rrs = [pool.tile([P, 1], dt, name=f"rr{k}") for k in range(4)]
        for k, ((w_ap, sgn, ll_ap), rrk) in enumerate(zip(terms, rrs)):
            ejk = pool.tile([P, F], dt, name=f"ej{k}")
            dve(nc.vector.scalar_tensor_tensor(out=ejk[:], in0=w_ap,
                                               scalar=sgn, in1=ll_ap,
                                               op0=Alu.mult, op1=Alu.mult,
                                               accum_out=rrk[:]))

        # Partition sum with 1/N folded in; four matmuls accumulate the
        # per-term columns into one PSUM scalar as each term lands.
        Fp = psum.tile([P, 1], dt)
        for k, rrk in enumerate(rrs):
            nc.tensor.matmul(Fp[:], onesN[:], rrk[:],
                             start=(k == 0), stop=(k == 3))
        res = pool.tile([1, 1], dt)
        dve(nc.vector.tensor_scalar_mul(res[:], Fp[0:1, 0:1], 1.0))
        nc.sync.dma_start(out.ap(), res[:])

        for prev, nxt in zip(dve_chain, dve_chain[1:]):
            add_dep_helper(nxt.ins, prev.ins, sync=False,
                           reason="forced DVE stream order")
        for prev, nxt in zip(act_chain, act_chain[1:]):
            add_dep_helper(nxt.ins, prev.ins, sync=False,
                           reason="forced ACT stream order")

    # ---- entry/exit block surgery (post-scheduling, pre-compile) ----
    fn = nc.m.functions[0]
    b0, b1, b2 = fn.blocks[0], fn.blocks[1], fn.blocks[2]
    Pool = mybir.EngineType.Pool
    SP = mybir.EngineType.SP

    # Framework Pool constant memsets: off the barrier's critical path.
    movers = [i for i in b0.instructions
              if type(i).__name__ == "InstMemset" and i.engine == Pool]
    for i in movers:
        b0.instructions.remove(i)
    idx = next(k for k, i in enumerate(b1.instructions) if i.engine == Pool)
    b1.instructions[idx:idx] = movers

    # Input DMA ahead of SP's pre-barrier drain.
    dmai = dma_in.ins
    b1.instructions.remove(dmai)
    sp_idx = next(k for k, i in enumerate(b0.instructions) if i.engine == SP)
    b0.instructions.insert(sp_idx, dmai)

    # Slim teardown: sem clear moves to program start (idle Pool, before its
    # pre-barrier drain); both epilogue barrier rounds removed -- engines
    # drain themselves, SP still waits on the DMA completion sems first.
    isa = [i for i in b2.instructions if type(i).__name__ == "InstISA"]
    assert len(isa) == 1
    if isa[0].sync_info is not None:
        isa[0].sync_info.on_wait = []
        isa[0].sync_info.on_update = []
    b2.instructions.remove(isa[0])
    pool_idx = next(k for k, i in enumerate(b0.instructions)
                    if i.engine == Pool)
    b0.instructions.insert(pool_idx, isa[0])
    keep = []
    drained = set()
    for i in b2.instructions:
        tn = type(i).__name__
        if tn == "InstEventSemaphore":
            si = i.sync_info
            if si is not None and si.on_wait and \
                    si.on_wait[0].ant_name.startswith("DMAHW") and \
                    not si.on_update:
                keep.append(i)
            continue
        if tn == "InstDrain":
            if i.engine in drained:
                continue
            drained.add(i.engine)
            if i.sync_info is not None:
                i.sync_info.on_update = []
                i.sync_info.on_wait = []
            keep.append(i)
            continue
        keep.append(i)
    b2.instructions[:] = keep

    nc.compile()
    return nc


def _build_nc_full():
    """Original closed-form O(N) kernel handling nonzero u buffers."""
    from contextlib import ExitStack

    import concourse.bacc as bacc
    import concourse.mybir as mybir
    import concourse.tile as tile
    from concourse.tile_rust import add_dep_helper

    dt = mybir.dt.float32
    Act = mybir.ActivationFunctionType
    Alu = mybir.AluOpType
    Ax = mybir.AxisListType

    nc = bacc.Bacc(
        "TRN2",
        target_bir_lowering=False,
        debug=False,
        enable_asserts=False,
        num_devices=NCORES,
    )
    # Packed input: columns [f | t | up | ua | q], each P x F.
    inp = nc.dram_tensor("inp", [P, 5 * F], dt, kind="ExternalInput")
    out = nc.dram_tensor("out", [1, 1], dt, kind="ExternalOutput")

    dve_chain = []
    pool_chain = []

    def dve(inst):
        dve_chain.append(inst)
        return inst

    def plq(inst):
        pool_chain.append(inst)
        return inst

    with tile.TileContext(nc) as tc, ExitStack() as ctx:
        pool = ctx.enter_context(tc.tile_pool(name="sb", bufs=1))
        psum = ctx.enter_context(tc.tile_pool(name="ps", bufs=1, space="PSUM"))

        x = pool.tile([P, 4 * F], dt)   # [f | t | up | ua]
        L = pool.tile([P, 4 * F], dt)   # [f | a | q | qc] -> packed Ln input
        nc.sync.dma_start(x[:, 0 : 2 * F], inp.ap()[:, 0 : 2 * F])
        nc.sync.dma_start(L[:, 2 * F : 3 * F], inp.ap()[:, 4 * F : 5 * F])
        nc.sync.dma_start(x[:, 2 * F : 4 * F], inp.ap()[:, 2 * F : 4 * F])
        f = x[:, 0 * F : 1 * F]
        t = x[:, 1 * F : 2 * F]
        upua = x[:, 2 * F : 4 * F]
        qL = L[:, 2 * F : 3 * F]

        ones128 = pool.tile([P, P], dt)
        nc.gpsimd.memset(ones128[:], 1.0 / N)
        consts = pool.tile([P, 2], dt)  # [1.0, 1e-12]
        dve(nc.vector.memset(consts[:, 0:1], 1.0))
        dve(nc.vector.memset(consts[:, 1:2], 1e-12))
        facA = pool.tile([P, 2], dt)    # [2*GAMMA, GAMMA] on mean moments
        dve(nc.vector.memset(facA[:, 0:1], 2 * GAMMA))
        dve(nc.vector.memset(facA[:, 1:2], GAMMA))
        facB = pool.tile([P, 3], dt)
        dve(nc.vector.memset(facB[:, 0:1], 2 * GAMMA))
        dve(nc.vector.memset(facB[:, 1:2], GAMMA))
        dve(nc.vector.memset(facB[:, 2:3], GAMMA))

        warm = pool.tile([P, 1], dt)
        nc.scalar.activation(out=warm[:], in_=consts[:, 0:1], func=Act.Ln,
                             bias=consts[:, 1:2], scale=1.0)

        plq(nc.gpsimd.tensor_copy(L[:, 0:F], f))
        plq(nc.gpsimd.tensor_scalar(out=L[:, 3 * F : 4 * F], in0=qL,
                                    scalar1=-1.0, scalar2=1.0,
                                    op0=Alu.mult, op1=Alu.add))
        nc.scalar.activation(out=L[:, F : 2 * F], in_=f, func=Act.Identity,
                             bias=consts[:, 0:1], scale=-1.0)
        a = L[:, F : 2 * F]
        LL = pool.tile([P, 4 * F], dt)
        nc.scalar.activation(out=LL[:], in_=L[:], func=Act.Ln,
                             bias=consts[:, 1:2], scale=1.0)
        nc.scalar.activation(out=L[:, 2 * F : 4 * F], in_=L[:, 0 : 2 * F],
                             func=Act.Identity, bias=0.0, scale=-1.0)

        r = pool.tile([P, 5], dt)
        tf = pool.tile([P, F], dt)
        j1 = pool.tile([P, F], dt)
        j2 = pool.tile([P, F], dt)
        dve(nc.vector.reduce_sum(
            out=r[:, 0:5:4],
            in_=x[:, 0 : 2 * F].rearrange("p (k f) -> p k f", k=2),
            axis=Ax.X))
        dve(nc.vector.scalar_tensor_tensor(out=j1[:], in0=f, scalar=1.0, in1=f,
                                           op0=Alu.mult, op1=Alu.mult,
                                           accum_out=r[:, 1:2]))
        dve(nc.vector.scalar_tensor_tensor(out=tf[:], in0=t, scalar=1.0, in1=f,
                                           op0=Alu.mult, op1=Alu.mult,
                                           accum_out=r[:, 2:3]))
        dve(nc.vector.scalar_tensor_tensor(out=j2[:], in0=tf[:], scalar=1.0,
                                           in1=f, op0=Alu.mult, op1=Alu.mult,
                                           accum_out=r[:, 3:4]))

        RpA = psum.tile([P, 2], dt)
        nc.tensor.matmul(RpA[:], ones128[:], r[:, 0:2], start=True, stop=True)
        RpB = psum.tile([P, 3], dt)
        nc.tensor.matmul(RpB[:], ones128[:], r[:, 2:5], start=True, stop=True)
        CA = pool.tile([P, 2], dt)      # [cS1, cS2]
        dve(nc.vector.tensor_mul(CA[:], RpA[:], facA[:]))
        CB = pool.tile([P, 3], dt)      # [cP1, cP2, cP0]
        dve(nc.vector.tensor_mul(CB[:], RpB[:], facB[:]))

        SPK = pool.tile([P, 2 * F], dt)
        Sterm = pool.tile([P, F], dt)
        Sp = pool.tile([P, F], dt)
        dve(nc.vector.tensor_scalar(out=Sterm[:], in0=a, scalar1=GAMMA,
                                    scalar2=CA[:, 0:1], op0=Alu.mult,
                                    op1=Alu.add))
        rnp = pool.tile([1, 1], dt)
        dve(nc.vector.reciprocal(rnp[:], CB[0:1, 2:3]))
        dve(nc.vector.tensor_mul(Sp[:], a, Sterm[:]))
        rnp9 = pool.tile([1, 1], dt)
        dve(nc.vector.tensor_scalar_mul(rnp9[:], rnp[:], 1.0 - GAMMA))
        dve(nc.vector.tensor_scalar_add(SPK[:, 0:F], Sp[:], CA[:, 1:2]))
        Pterm = pool.tile([P, F], dt)
        Pp = pool.tile([P, F], dt)
        plq(nc.gpsimd.tensor_scalar(out=Pterm[:], in0=a, scalar1=CB[:, 2:3],
                                    scalar2=CB[:, 0:1], op0=Alu.mult,
                                    op1=Alu.add))
        plq(nc.gpsimd.tensor_mul(Pp[:], a, Pterm[:]))
        plq(nc.gpsimd.tensor_scalar_add(SPK[:, F : 2 * F], Pp[:], CB[:, 1:2]))
        m12 = pool.tile([P, 2 * F], dt)
        plq(nc.gpsimd.tensor_mul(m12[:], upua, SPK[:]))

        uan = pool.tile([P, F], dt)
        dve(nc.vector.scalar_tensor_tensor(out=uan[:], in0=x[:, 3 * F : 4 * F],
                                           scalar=1.0 - GAMMA, in1=SPK[:, 0:F],
                                           op0=Alu.mult, op1=Alu.add))
        den = pool.tile([P, F], dt)
        dve(nc.vector.tensor_mul(den[:], uan[:], uan[:]))
        rec = pool.tile([P, F], dt)
        dve(nc.vector.reciprocal(rec[:], den[:]))
        rec_t = pool.tile([P, F], dt)
        plq(nc.gpsimd.tensor_mul(rec_t[:], t, rec[:]))

        rr = pool.tile([P, 2], dt)  # [nat, adv]
        ej = pool.tile([P, 4 * F], dt)
        dve(nc.vector.scalar_tensor_tensor(out=ej[:], in0=L[:], scalar=1.0,
                                           in1=LL[:], op0=Alu.mult,
                                           op1=Alu.mult,
                                           accum_out=rr[:, 1:2]))

        num = pool.tile([P, F], dt)
        dve(nc.vector.tensor_sub(num[:], m12[:, 0:F], m12[:, F : 2 * F]))
        cj = pool.tile([P, F], dt)
        dve(nc.vector.scalar_tensor_tensor(out=cj[:], in0=num[:], scalar=1.0,
                                           in1=rec_t[:], op0=Alu.mult,
                                           op1=Alu.mult,
                                           accum_out=rr[:, 0:1]))

        Fp = psum.tile([P, 2], dt)
        nc.tensor.matmul(Fp[:], ones128[:], rr[:], start=True, stop=True)
        v1 = pool.tile([1, 1], dt)
        dve(nc.vector.tensor_mul(v1[:], Fp[0:1, 0:1], rnp9[:]))
        res = pool.tile([1, 1], dt)
        dve(nc.vector.tensor_tensor(out=res[:], in0=Fp[0:1, 1:2], in1=v1[:],
                                    op=Alu.add))
        nc.sync.dma_start(out.ap(), res[:])

        for prev, nxt in zip(dve_chain, dve_chain[1:]):
            add_dep_helper(nxt.ins, prev.ins, sync=False,
                           reason="forced DVE stream order")
        for prev, nxt in zip(pool_chain, pool_chain[1:]):
            add_dep_helper(nxt.ins, prev.ins, sync=False,
                           reason="forced Pool stream order")

    nc.compile()
    return nc


def _get_nc():
    global _NC_FAST
    if _NC_FAST is None:
        _NC_FAST = _build_nc_fast()
    return _NC_FAST


def _get_nc_full():
    global _NC_FULL
    if _NC_FULL is None:
        _NC_FULL = _build_nc_full()
    return _NC_FULL


def _pack_fast(y_pred, y_pred_adv):
    f = np.asarray(y_pred, dtype=np.float32).reshape(P, F)
    q = np.asarray(y_pred_adv, dtype=np.float32).reshape(P, F)
    return np.ascontiguousarray(np.concatenate([f, q], axis=1))


def _pack_full(y_pred, y_pred_adv, y_true, ua, up):
    f = np.asarray(y_pred, dtype=np.float32).reshape(-1)
    q = np.asarray(y_pred_adv, dtype=np.float32).reshape(-1)
    t = (np.asarray(y_true).reshape(-1) == 1).astype(np.float32)
    packed = np.stack([f, t, up, ua, q]).reshape(5, P, F).transpose(1, 0, 2)
    return np.ascontiguousarray(packed.reshape(P, 5 * F))


def _run(nc, inp, trace):
    import time

    from concourse.bass_utils import run_bass_kernel_spmd

    in_maps = [{"inp": inp} for _ in range(NCORES)]
    # The fleet occasionally reports a transient NRT_EXEC_UNIT_UNRECOVERABLE
    # left over from an earlier crashed process; retry a couple of times.
    last_exc = None
    for attempt in range(3):
        try:
            return run_bass_kernel_spmd(nc, in_maps,
                                        core_ids=list(range(NCORES)),
                                        trace=trace)
        except Exception as exc:  # noqa: BLE001
            last_exc = exc
            time.sleep(10 * (attempt + 1))
    raise last_exc


def kernel(y_pred, y_pred_adv, u_all, u_pos, y_true, index_s, _trace=False):
    idx = np.asarray(index_s).reshape(-1).astype(np.int64)
    ua = np.asarray(u_all, dtype=np.float32).reshape(-1)[idx]
    up = np.asarray(u_pos, dtype=np.float32).reshape(-1)[idx]
    if not (ua.any() or up.any()):
        # nat_loss is identically zero (see header) -> adv-only fast kernel
        nc = _get_nc()
        inp = _pack_fast(y_pred, y_pred_adv)
    else:
        nc = _get_nc_full()
        inp = _pack_full(y_pred, y_pred_adv, y_true, ua, up)
    bres = _run(nc, inp, _trace)
    val = np.asarray(bres.results[0]["out"], dtype=np.float32).reshape(())
    if _trace:
        return val, bres
    return val
